# revision 18
# baseline (speedup 1.0000x reference)
"""DCE loss kernel for Trainium2 (8 NeuronCores, SPMD via bass).

loss[b] = cnt[c_b] * log(sum_p exp(-dist[b,p])) + sum_{p in class(b)} dist[b,p]

Device computes, per core (protos strided-sharded after a class sort):
  - u[b, p] = |x_b - p_p|^2 via two accumulating matmuls (fp32r)
  - dist = Sqrt(u + x2[b])   (ACT, bias per-partition, reads PSUM)
  - per-class column-range sums of dist (DVE reduce_sum)  -> msum
  - exp(-dist) in-place (ACT) + row sums (DVE)            -> onec
Host does the sort/pad/shard prep, final log/gather/combine, and unsort.
"""

import sys

import numpy as np

sys.path.insert(0, "/opt/trn_rl_repo")

import concourse.bass as bass  # noqa: E402
import concourse.bacc as bacc  # noqa: E402
import concourse.mybir as mybir  # noqa: E402
import concourse.tile as tile  # noqa: E402
from concourse.bass_utils import run_bass_kernel_spmd  # noqa: E402
from concourse.tile_rust import add_dep_helper  # noqa: E402

F32 = mybir.dt.float32
F16 = mybir.dt.float16
BF16 = mybir.dt.bfloat16
F32R = mybir.dt.float32r
ACT = mybir.ActivationFunctionType

NCORES = 8
PADV = 100.0  # pad-prototype first coordinate (rest zeros)

# knobs (test.py pokes these)
TRACE = False
SIM = False
LAST_EXEC_NS = None
LAST_RESULTS = None

_BUILD_CACHE = {}

# variant knobs (analyze.py / test.py sweep these)
DIST16 = False  # fp16 dist + bf16 exp output (False: fp32 everywhere)


def _build_program(B, W, ops, grp, reps=0):
    """Build the SPMD bass program. ops: tuple of (bt, sc, ec) masked-sum ops.
    reps>0 wraps the compute body in a For_i loop (benchmark variants)."""
    NBT = B // 128
    S = len(ops)
    nc = bacc.Bacc("TRN2", target_bir_lowering=False, debug=False)

    # matmul operands ride in one packed tensor: [pT | p2 | ones | xT].
    # Loaded as a small leading DMA (pT/p2/ones) + 4 xT column chunks so
    # the first matmuls can start before the whole 2MB xT lands.
    TW = B + 2 * W + 128
    pk_d = nc.dram_tensor("pack", [128, TW], F32R, kind="ExternalInput").ap()
    x2_d = nc.dram_tensor("x2c", [128, NBT], F32, kind="ExternalInput").ap()
    onec_d = nc.dram_tensor("onec", [128, NBT], F32, kind="ExternalOutput").ap()
    msum_d = nc.dram_tensor("msum", [128, max(S, 1)], F32, kind="ExternalOutput").ap()

    # bank-aligned matmul chunks (each within one 2KB PSUM bank)
    chunks = [(c, min(c + 512, W)) for c in range(0, W, 512)]

    ops_by_bt = {}
    for slot, (bt, sc, ec) in enumerate(ops):
        ops_by_bt.setdefault(bt, []).append((slot, sc, ec))

    from contextlib import ExitStack

    with tile.TileContext(nc) as tc, ExitStack() as ctx:
        const_p = ctx.enter_context(tc.tile_pool(name="const", bufs=1))
        psum_p = ctx.enter_context(tc.tile_pool(name="psum", bufs=2, space="PSUM"))
        dist_p = ctx.enter_context(tc.tile_pool(name="dist", bufs=1))
        out_p = ctx.enter_context(tc.tile_pool(name="outs", bufs=1))

        pk_sb = const_p.tile([128, TW], F32R, tag="pack")
        head = 2 * W + 128
        nc.sync.dma_start(pk_sb[:, 0:head], pk_d[:, 0:head])
        xq = B // 4
        for q in range(4):
            nc.sync.dma_start(
                pk_sb[:, head + q * xq : head + (q + 1) * xq],
                pk_d[:, head + q * xq : head + (q + 1) * xq],
            )
        x2_sb = const_p.tile([128, NBT], F32, tag="x2")
        nc.sync.dma_start(x2_sb[:], x2_d[:])
        pT_sb = pk_sb[:, 0:W]
        p2_sb = pk_sb[0:1, W : 2 * W]
        ones_sb = pk_sb[0:1, 2 * W : 2 * W + 128]
        xT_sb = pk_sb[:, head : head + B]

        onec_sb = out_p.tile([128, NBT], F32, tag="onec")
        msum_sb = out_p.tile([128, max(S, 1)], F32, tag="msum")

        dist_sb = dist_p.tile([128, grp * W], F16 if DIST16 else F32, tag="dist")

        from contextlib import nullcontext

        loop_cm = tc.For_i(0, reps, 1) if reps else nullcontext()
        act_chain = []
        with loop_cm:
            body(nc, tc, NBT, grp, W, chunks, ops_by_bt, act_chain,
                 xT_sb, pT_sb, p2_sb, ones_sb, x2_sb, dist_sb, onec_sb,
                 msum_sb, psum_p)

        # pin the ACT instruction order so sqrt/exp phases don't interleave
        # (a sqrt<->exp table switch costs ~1.3us each)
        for a, b in zip(act_chain, act_chain[1:]):
            add_dep_helper(b.ins, a.ins, sync=False, reason="act phase order")

        nc.sync.dma_start(onec_d[:], onec_sb[:])
        nc.sync.dma_start(msum_d[:], msum_sb[:])

    nc.compile()
    return nc


def body(nc, tc, NBT, grp, W, chunks, ops_by_bt, act_chain, xT_sb, pT_sb,
         p2_sb, ones_sb, x2_sb, dist_sb, onec_sb, msum_sb, psum_p):
    ACT = mybir.ActivationFunctionType
    for g in range(NBT // grp):
            for j in range(grp):
                bt = g * grp + j
                u = psum_p.tile([128, W], F32, name="u", tag="u")
                for c0, c1 in chunks:
                    nc.tensor.matmul(
                        u[:, c0:c1],
                        lhsT=xT_sb[:, bt * 128 : (bt + 1) * 128],
                        rhs=pT_sb[:, c0:c1],
                        start=True,
                        stop=False,
                    )
                for c0, c1 in chunks:
                    nc.tensor.matmul(
                        u[:, c0:c1],
                        lhsT=ones_sb,
                        rhs=p2_sb[:, c0:c1],
                        start=False,
                        stop=True,
                    )
                dsl = dist_sb[:, j * W : (j + 1) * W]
                i_sqrt = nc.scalar.activation(
                    dsl, u[:, 0:W], ACT.Sqrt, bias=x2_sb[:, bt : bt + 1], scale=1.0
                )
                act_chain.append(i_sqrt)
                for slot, sc, ec in ops_by_bt.get(bt, []):
                    nc.vector.reduce_sum(
                        msum_sb[:, slot : slot + 1],
                        dist_sb[:, j * W + sc : j * W + ec],
                        axis=mybir.AxisListType.X,
                    )
            for j in range(grp):
                bt = g * grp + j
                dsl = dist_sb[:, j * W : (j + 1) * W]
                # in-place exp; with fp16 dist, write bf16 over the same
                # bytes (bf16 avoids fp16 subnormal underflow at exp(-25))
                eout = dsl.bitcast(BF16) if DIST16 else dsl
                i_exp = nc.scalar.activation(eout, dsl, ACT.Exp, scale=-1.0)
                act_chain.append(i_exp)
                nc.vector.reduce_sum(
                    onec_sb[:, bt : bt + 1], eout, axis=mybir.AxisListType.X
                )


class _Prep:
    """Host-side preparation: program + per-core inputs + result assembly."""

    def __init__(self, nc, in_maps, assemble):
        self.nc = nc
        self.in_maps = in_maps
        self.assemble = assemble


def prepare(feature, label, proto_features, proto_labels, reps=0):
    x = np.asarray(feature, dtype=np.float32)
    lab = np.asarray(label).astype(np.int64)
    pf = np.asarray(proto_features, dtype=np.float32)
    plab = np.asarray(proto_labels).astype(np.int64)

    B, D = x.shape
    P = pf.shape[0]
    assert D == 128 and B % 128 == 0
    NBT = B // 128

    # ---------------- host prep: sort protos by class, pad to mult of 8 ----
    order_p = np.argsort(plab, kind="stable")
    plab_s = plab[order_p]
    classes = np.unique(plab_s)
    nclass_max = int(plab_s.max()) + 1 if P else 1

    cnt = np.bincount(plab, minlength=nclass_max).astype(np.int64)

    blocks = []
    cls_grange = {}  # class -> (gstart, gend_padded)
    npad = np.zeros(nclass_max, dtype=np.int64)
    pos = 0
    pad_row = np.zeros((1, D), dtype=np.float32)
    pad_row[0, 0] = PADV
    for c in classes:
        lo = np.searchsorted(plab_s, c, "left")
        hi = np.searchsorted(plab_s, c, "right")
        idx = order_p[lo:hi]
        n = len(idx)
        npc = ((n + 7) // 8) * 8
        blocks.append(pf[idx])
        if npc > n:
            blocks.append(np.repeat(pad_row, npc - n, axis=0))
        npad[c] = npc - n
        cls_grange[int(c)] = (pos, pos + npc)
        pos += npc
    # global pad so W = pos/8 is even (fp32r matmul needs an even moving dim)
    gpad = (-pos) % (2 * NCORES)
    if gpad:
        blocks.append(np.repeat(pad_row, gpad, axis=0))
        pos += gpad
    padded = np.concatenate(blocks, axis=0)  # [pos, D], pos % 16 == 0
    W = pos // NCORES
    assert pos % NCORES == 0 and W % 2 == 0

    # strided shard: core k holds sorted-padded protos k::8
    pT_cores = []
    p2_cores = []
    for k in range(NCORES):
        sh = padded[k::NCORES]  # [W, D]
        pT_cores.append(np.ascontiguousarray(-2.0 * sh.T, dtype=np.float32))
        p2_cores.append((sh * sh).sum(1, dtype=np.float32))

    # ---------------- host prep: sort features by class -------------------
    order_b = np.argsort(lab, kind="stable")
    xs = x[order_b]
    lab_s = lab[order_b]
    xT = np.ascontiguousarray(xs.T, dtype=np.float32)  # [128, B]
    x2 = (xs * xs).sum(1, dtype=np.float32)  # [B]
    x2c = np.ascontiguousarray(x2.reshape(NBT, 128).T, dtype=np.float32)

    # ---------------- masked-op schedule (uniform across cores) -----------
    ops = []
    op_meta = []  # (bt, class, row_lo, row_hi)  rows local to bt
    for c in classes:
        g0, g1 = cls_grange[int(c)]
        sc, ec = g0 // NCORES, g1 // NCORES
        bs = int(np.searchsorted(lab_s, c, "left"))
        be = int(np.searchsorted(lab_s, c, "right"))
        if bs == be:
            continue
        for bt in range(bs // 128, (be + 127) // 128):
            ops.append((bt, sc, ec))
            rlo = max(bs, bt * 128) - bt * 128
            rhi = min(be, (bt + 1) * 128) - bt * 128
            op_meta.append((bt, int(c), rlo, rhi))
    ops = tuple(ops)
    S = len(ops)

    grp = 16 if NBT % 16 == 0 else NBT

    key = (B, W, ops, grp, reps, DIST16)
    if key not in _BUILD_CACHE:
        _BUILD_CACHE[key] = _build_program(B, W, ops, grp, reps)
    nc = _BUILD_CACHE[key]

    # packed matmul-operand tensor: [pT | p2(row0) | ones(row0) | xT]
    TW = B + 2 * W + 128
    in_maps = []
    for k in range(NCORES):
        pack = np.zeros((128, TW), dtype=np.float32)
        pack[:, 0:W] = pT_cores[k]
        pack[0, W : 2 * W] = p2_cores[k]
        pack[0, 2 * W : 2 * W + 128] = 1.0
        pack[:, 2 * W + 128 : 2 * W + 128 + B] = xT
        in_maps.append({"pack": pack, "x2c": x2c})

    def assemble(results):
        onec = np.zeros((128, NBT), dtype=np.float64)
        msum = np.zeros((128, max(S, 1)), dtype=np.float64)
        for r in results:
            onec += r["onec"].astype(np.float64)
            msum += r["msum"].astype(np.float64)

        one = onec.T.reshape(B)  # one[b], b in sorted order
        log_one = np.log(one)

        masked = np.zeros(B, dtype=np.float64)
        for slot, (bt, c, rlo, rhi) in enumerate(op_meta):
            masked[bt * 128 + rlo : bt * 128 + rhi] += msum[rlo:rhi, slot]

        # subtract pad-proto contributions (pad = (PADV, 0, ..., 0))
        d_pad = np.sqrt(x2.astype(np.float64) + PADV * PADV - 2.0 * PADV * xs[:, 0])
        masked -= npad[lab_s] * d_pad

        loss_s = cnt[lab_s] * log_one + masked
        loss = np.empty(B, dtype=np.float64)
        loss[order_b] = loss_s
        return loss.astype(np.float32)

    return _Prep(nc, in_maps, assemble)


def kernel(feature, label, proto_features, proto_labels):
    global LAST_EXEC_NS, LAST_RESULTS
    p = prepare(feature, label, proto_features, proto_labels)
    nc, in_maps = p.nc, p.in_maps

    if SIM:
        from concourse.bass_interp import CoreSim

        results = []
        for k in range(NCORES):
            sim = CoreSim(nc, trace=False)
            for name, arr in in_maps[k].items():
                sim.tensor(name)[:] = arr
            sim.simulate(check_with_hw=False)
            results.append(
                {
                    "onec": sim.tensor("onec").copy(),
                    "msum": sim.tensor("msum").copy(),
                }
            )
        LAST_EXEC_NS = None
    else:
        res = run_bass_kernel_spmd(nc, in_maps, list(range(NCORES)), trace=TRACE)
        results = res.results
        LAST_EXEC_NS = res.exec_time_ns
    LAST_RESULTS = results
    return p.assemble(results)
